# revision 51
# baseline (speedup 1.0000x reference)
"""Trainium2 8-core attention kernel for nn_Attention_8409545965959.

Reference computation (B=4, N=2048, C=1024, H=16 heads, Dh=64):
    qkv = x @ Wqkv; q,k,v per head
    att = softmax(where(mask>0, -1e7, q @ k^T / sqrt(Dh)))
    out = (att @ v) @ Wproj + bproj

Masked keys contribute exactly zero to the softmax, so K/V are compacted
host-side to the unmasked tokens of each batch, padded per batch to a
multiple of 128 (padded positions re-masked on device via the exp bias).

Sharding: tensor-parallel on heads (2 heads/core), per-q-block AllToAlls
reshard the attention output to interleaved sequence ownership (core c
owns q rows qb*512 + c*64 + [0:64) of every q-block), and each core
computes full output rows for its slice (row-parallel proj).

Schedule: one globally software-pipelined slot stream over all
(batch, q-block, k-chunk) attention steps. Per slot: the score pair for
slot s, the AV pair and denominator quad for slot s-2, with QKV chains
for the next batch, projection chains, and normalization woven in as
cost-paced filler so neither the PE nor the ScalarE exp pipeline ever
idles (idle >3.4us re-throttles the PE clock to 1.2GHz via HAM).
Denominator ones-matmuls use 4-way column tiling (positions 0/32/64/96
by k-chunk parity) so two D chains stream concurrently.

kernel(**inputs) accepts the full unsharded inputs and returns the full
[4, 2048, 1024] float32 output.
"""

import sys
import types

import numpy as np
import ml_dtypes

# If a caller enables BASS_TRACE without the axon NTFF profiling hook
# installed, concourse's trace path would fail importing
# antenv.axon_hooks. Provide a no-op fallback (never overrides a real
# module) so tracing degrades gracefully instead of crashing.
try:
    import antenv.axon_hooks  # noqa: F401
except ImportError:
    try:
        import antenv

        _ah = types.ModuleType("antenv.axon_hooks")
        _ah._hook = None
        _ah.set_axon_ntff_profile_hook = lambda h: setattr(_ah, "_hook", h)
        _ah.get_axon_ntff_profile_hook = lambda: _ah._hook
        sys.modules["antenv.axon_hooks"] = _ah
        antenv.axon_hooks = _ah
    except ImportError:
        pass

import concourse.bass as bass
import concourse.mybir as mybir
import concourse.tile as tile
from concourse import bacc
from concourse.bass_utils import run_bass_kernel_spmd

B = 4
N = 2048
C = 1024
H = 16
NCORES = 8
DH = C // H            # 64
HPC = H // NCORES      # 2 heads per core -> 128 channels/core
CPC = HPC * DH         # 128
ROWS = B * N           # 8192
QB = 512               # q block (one PSUM bank of f32)
KCH = 128              # k chunk (partitions)
NQB = N // QB          # 4
CC = C // 128          # 8 contraction chunks
SCALE = DH ** -0.5     # 0.125
MASK_BIAS = -30000.0
QS = QB // NCORES      # 64 q rows per dest core per q-block
LAG = 2                # av/d lag behind se in the slot pipeline

DT = mybir.dt.float32
BF = mybir.dt.bfloat16
NPBF = ml_dtypes.bfloat16

_CACHE: dict = {}
LAST_RESULTS = None

# warm-ish cost estimates (ns) for filler pacing. Measured exp cadence is
# ~1.33us per [128,1024] tile; attention PE per slot ~540ns.
FILLER_BUDGET = 700


def _build(nkcs):
    """nkcs = per-batch number of 128-row k-chunks after compaction."""
    nks = [nkc * KCH for nkc in nkcs]
    koffs = [0]
    for nk in nks:
        koffs.append(koffs[-1] + nk)
    totk = koffs[-1]
    moffs = [0]
    for nkc in nkcs:
        moffs.append(moffs[-1] + nkc)
    totkc = moffs[-1]
    max_nk = max(nks)

    nc = bacc.Bacc("TRN2", target_bir_lowering=False, debug=False, num_devices=NCORES)

    xT = nc.dram_tensor("xT", [C, ROWS], BF, kind="ExternalInput")
    xTk = nc.dram_tensor("xTk", [C, totk], BF, kind="ExternalInput")
    wq = nc.dram_tensor("wq", [C, CPC], BF, kind="ExternalInput")
    wk = nc.dram_tensor("wk", [C, CPC], BF, kind="ExternalInput")
    wv = nc.dram_tensor("wv", [C, CPC], BF, kind="ExternalInput")
    wp = nc.dram_tensor("wp", [C, C], BF, kind="ExternalInput")
    bvec = nc.dram_tensor("bvec", [128, CC], DT, kind="ExternalInput")
    mb = nc.dram_tensor("mb", [128, totkc], DT, kind="ExternalInput")
    sel4 = nc.dram_tensor("sel4", [128, 128], DT, kind="ExternalInput")
    out_ext = nc.dram_tensor("out", [C, B * NQB * QS], BF, kind="ExternalOutput")

    # k blocks for the K^T qkv matmuls (moving dim <= 512), per batch
    kblocks = []
    for nk in nks:
        blocks = []
        pos = 0
        while pos < nk:
            w = min(QB, nk - pos)
            blocks.append((pos, w))
            pos += w
        kblocks.append(blocks)

    # global slot list
    slots = [(b, qb, kc) for b in range(B) for qb in range(NQB)
             for kc in range(nkcs[b])]
    idx_of = {s: i for i, s in enumerate(slots)}
    batch_start = {b: idx_of[(b, 0, 0)] for b in range(B)}

    with tile.TileContext(nc) as tc:
        with (
            tc.tile_pool(name="consts", bufs=1) as consts,
            tc.tile_pool(name="xpool", bufs=7) as xpool,
            tc.tile_pool(name="kpool", bufs=2) as kpool,
            tc.tile_pool(name="qkpool", bufs=2) as qkpool,
            tc.tile_pool(name="vpool", bufs=3) as vpool,
            tc.tile_pool(name="epool", bufs=6) as epool,
            tc.tile_pool(name="npool", bufs=2) as npool,
            tc.tile_pool(name="opool", bufs=2) as opool,
            tc.tile_pool(name="dram", bufs=1, space="DRAM") as dram,
            tc.tile_pool(name="s_ps", bufs=2, space="PSUM") as s_ps,
            tc.tile_pool(name="o_ps", bufs=1, space="PSUM") as o_ps,
            tc.tile_pool(name="d_ps", bufs=1, space="PSUM") as d_ps,
            tc.tile_pool(name="aux_ps", bufs=2, space="PSUM") as aux_ps,
        ):
            # ---- persistent constants / weights
            wq_sb = consts.tile([128, CC, CPC], BF)
            wk_sb = consts.tile([128, CC, CPC], BF)
            wv_sb = consts.tile([128, CC, CPC], BF)
            wp_sb = consts.tile([128, CC, C], BF)
            bias_sb = consts.tile([128, CC], DT)
            mb_sb = consts.tile([128, totkc], DT)
            sel4_sb = consts.tile([128, 128], DT)
            ones_sb = consts.tile([128, 1], BF)
            warm_sb = consts.tile([128, QB], BF)
            # persistent D staging: memset once; only rows {0,32,64,96} are
            # ever written, so the sel4 matmul never sees uninitialized data
            dstage = consts.tile([128, QB], DT)
            nc.vector.memset(dstage[:], 0.0)
            nc.vector.memset(ones_sb[:], 1.0)
            nc.vector.memset(warm_sb[:], 0.0)
            # pre-load the ScalarE exp spline tables (~2.7us) during the
            # input DMA wait instead of before the first real exp
            nc.scalar.activation(
                warm_sb[0:1, 0:8], warm_sb[0:1, 0:8],
                mybir.ActivationFunctionType.Exp,
            )
            # PE warm-up: dummy matmuls cover the initial DMA wait
            warm_ps = aux_ps.tile([128, QB], DT, name="warm", tag="aux")
            for _ in range(16):
                nc.tensor.matmul(
                    warm_ps[:], warm_sb[:, 0:128], warm_sb[:], start=True, stop=True
                )

            # input DMAs, ordered so the first k/q chains can start asap
            nc.sync.dma_start(wk_sb[:], wk.rearrange("(cc p) m -> p cc m", p=128))

            # AllToAll bounce buffers: one per batch; dest core i gets q rows
            # qb*512 + i*64 + [0:64) of every q-block (interleaved ownership).
            # Batch 3 is split in two (qb 0-1 / qb 2-3) so the tail transfer
            # halves and its proj can start during the final collective.
            a2a_in = [
                dram.tile([NCORES, 128, NQB * QS], BF, name=f"a2a_in{b}",
                          tag=f"a2a_in{b}") for b in range(B - 1)
            ]
            a2a_out = [
                dram.tile([NCORES, 128, NQB * QS], BF, name=f"a2a_out{b}",
                          tag=f"a2a_out{b}") for b in range(B - 1)
            ]
            a2a_in3 = [
                dram.tile([NCORES, 128, 2 * QS], BF, name=f"a2a_in3{h}",
                          tag=f"a2a_in3{h}") for h in range(2)
            ]
            a2a_out3 = [
                dram.tile([NCORES, 128, 2 * QS], BF, name=f"a2a_out3{h}",
                          tag=f"a2a_out3{h}") for h in range(2)
            ]

            # startup alignment: absorb inter-core launch stagger on the
            # collective engine before real collectives hit the critical path
            align_in = dram.tile([2, 4], BF, name="align_in", tag="align_in")
            align_out = dram.tile([2, 4], BF, name="align_out", tag="align_out")
            nc.sync.dma_start(align_in[:], wq[0:2, 0:4])
            nc.gpsimd.collective_compute(
                "AllToAll",
                mybir.AluOpType.bypass,
                ins=[align_in.opt()],
                outs=[align_out.opt()],
                replica_groups=[list(range(NCORES))],
            )

            xbq_tiles = {}
            kb_tiles = {}
            qkv_state = {}

            def load_xbq(b, rb):
                # per-q-block x tiles in a deep ring: 1MB DMAs (near-peak
                # bandwidth) whose issue is never WAR-gated on the previous
                # batch (the ring is ~1.7 batches deep)
                xq = xpool.tile([128, CC, QB], BF, name=f"xq{b}_{rb}", tag="xbq")
                xbq_tiles[(b, rb)] = xq
                xs = xT[:, b * N:(b + 1) * N].rearrange("(cc p) n -> p cc n", p=128)
                nc.sync.dma_start(xq[:], xs[:, :, rb * QB:(rb + 1) * QB])

            def load_kb(b, part):
                # part 0: first 512-col block; part 1: the rest; part 2: all
                nk = nks[b]
                if part != 1:
                    kb = kpool.tile([128, CC, max_nk], BF, name=f"kb{b}", tag="kb")
                    kb_tiles[b] = kb
                kb = kb_tiles[b]
                ks = xTk[:, koffs[b]:koffs[b] + nk].rearrange(
                    "(cc p) n -> p cc n", p=128
                )
                if part == 0:
                    nc.sync.dma_start(kb[:, :, 0:QB], ks[:, :, 0:QB])
                elif part == 1:
                    if nk > QB:
                        nc.sync.dma_start(kb[:, :, QB:nk], ks[:, :, QB:nk])
                else:
                    nc.sync.dma_start(kb[:, :, 0:nk], ks[:, :, 0:nk])

            def emit_xb_load(b):
                load_kb(b, 2)
                for rb in range(NQB):
                    load_xbq(b, rb)

            def make_qkv_tiles(b):
                qT = qkpool.tile([128, N], BF, name=f"qT{b}", tag="qT")
                kT = qkpool.tile([128, max_nk], BF, name=f"kT{b}", tag="kT")
                vt = vpool.tile([128, max(nkcs), CPC], BF, name=f"vt{b}", tag="vt")
                qkv_state[b] = (qT, kT, vt)

            def q_unit(b, rb):
                def emit():
                    qT = qkv_state[b][0]
                    xq = xbq_tiles[(b, rb)]
                    ps = aux_ps.tile([128, QB], DT, name=f"psq{b}_{rb}", tag="aux")
                    for cc in range(CC):
                        nc.tensor.matmul(
                            ps[:],
                            wq_sb[:, cc, :],
                            xq[:, cc, :],
                            start=cc == 0,
                            stop=cc == CC - 1,
                        )
                    nc.vector.tensor_copy(qT[:, rb * QB:(rb + 1) * QB], ps[:])

                return emit

            def k_unit(b, pos, w):
                def emit():
                    kT = qkv_state[b][1]
                    kb = kb_tiles[b]
                    ps = aux_ps.tile([128, QB], DT, name=f"psk{b}_{pos}", tag="aux")
                    for cc in range(CC):
                        nc.tensor.matmul(
                            ps[:, 0:w],
                            wk_sb[:, cc, :],
                            kb[:, cc, pos:pos + w],
                            start=cc == 0,
                            stop=cc == CC - 1,
                        )
                    nc.vector.tensor_copy(kT[:, pos:pos + w], ps[:, 0:w])

                return emit

            def v_unit(b, rc):
                def emit():
                    vt = qkv_state[b][2]
                    kb = kb_tiles[b]
                    ps = aux_ps.tile([128, QB], DT, name=f"psv{b}_{rc}", tag="aux")
                    for cc in range(CC):
                        nc.tensor.matmul(
                            ps[:, 0:CPC],
                            kb[:, cc, rc * KCH:(rc + 1) * KCH],
                            wv_sb[:, cc, :],
                            start=cc == 0,
                            stop=cc == CC - 1,
                        )
                    nc.vector.tensor_copy(vt[:, rc, :], ps[:, 0:CPC])

                return emit

            # ---- attention slot pieces
            e_tiles = {}
            o_acc_t = {}
            d_acc_t = {}
            norm_state = {}

            def se_step(b, qb, kc):
                qT, kT, vt = qkv_state[b]
                e_tiles[(b, qb, kc)] = epool.tile(
                    [128, 2 * QB], BF, name=f"e{b}_{qb}_{kc}", tag="e"
                )
                s2 = s_ps.tile([128, 2 * QB], DT, name=f"s{b}_{qb}_{kc}", tag="s")
                nc.tensor.matmul(
                    s2[:, 0:QB],
                    kT[0:DH, kc * KCH:(kc + 1) * KCH],
                    qT[0:DH, qb * QB:(qb + 1) * QB],
                    start=True,
                    stop=True,
                    tile_position=(0, 0),
                )
                nc.tensor.matmul(
                    s2[:, QB:2 * QB],
                    kT[DH:2 * DH, kc * KCH:(kc + 1) * KCH],
                    qT[DH:2 * DH, qb * QB:(qb + 1) * QB],
                    start=True,
                    stop=True,
                    tile_position=(64, 0),
                )
                mcol = moffs[b] + kc
                nc.scalar.activation(
                    e_tiles[(b, qb, kc)][:],
                    s2[:],
                    mybir.ActivationFunctionType.Exp,
                    bias=mb_sb[:, mcol:mcol + 1],
                    scale=SCALE,
                )

            def av_step(b, qb, kc):
                nkc = nkcs[b]
                vt = qkv_state[b][2]
                if kc == 0:
                    o_acc_t[(b, qb)] = o_ps.tile(
                        [128, QB], DT, name=f"o{b}_{qb}", tag="o"
                    )
                o_acc = o_acc_t[(b, qb)]
                e2 = e_tiles[(b, qb, kc)]
                st = kc == 0
                sp = kc == nkc - 1
                nc.tensor.matmul(
                    o_acc[0:DH, :],
                    vt[:, kc, 0:DH],
                    e2[:, 0:QB],
                    start=st,
                    stop=sp,
                    tile_position=(0, 0),
                )
                nc.tensor.matmul(
                    o_acc[DH:2 * DH, :],
                    vt[:, kc, DH:2 * DH],
                    e2[:, QB:2 * QB],
                    start=st,
                    stop=sp,
                    tile_position=(0, 64),
                )

            def d_step(b, qb, kc):
                # 4-way column tiling: even kc chains at partitions 0/32,
                # odd kc chains at 64/96 -- two D chains stream concurrently
                nkc = nkcs[b]
                if kc == 0:
                    d_acc_t[(b, qb)] = d_ps.tile(
                        [128, QB], DT, name=f"d{b}_{qb}", tag="d"
                    )
                d_acc = d_acc_t[(b, qb)]
                e2 = e_tiles[(b, qb, kc)]
                par = kc % 2
                base = 64 * par
                st = kc < 2
                sp = kc + 2 >= nkc
                nc.tensor.matmul(
                    d_acc[base:base + 1, :],
                    ones_sb[:],
                    e2[:, 0:QB],
                    start=st,
                    stop=sp,
                    tile_position=(0, base),
                )
                nc.tensor.matmul(
                    d_acc[base + 32:base + 33, :],
                    ones_sb[:],
                    e2[:, QB:2 * QB],
                    start=st,
                    stop=sp,
                    tile_position=(0, base + 32),
                )

            def norm_a(b, qb):
                # free the PSUM accumulators quickly via DVE copies; only the
                # four live D rows move into the persistent staging tile
                osb = opool.tile([128, QB], DT, name=f"osb{b}_{qb}", tag="osb")
                nc.vector.tensor_copy(osb[:], o_acc_t[(b, qb)][:])
                d_acc = d_acc_t[(b, qb)]
                rows = (0, 32, 64, 96) if nkcs[b] > 1 else (0, 32)
                for r in rows:
                    nc.vector.tensor_copy(
                        dstage[r:r + 1, :], d_acc[r:r + 1, :]
                    )
                norm_state[(b, qb)] = osb

            def norm_b(b, qb):
                # one f32 matmul does the even+odd chain add AND the
                # partition broadcast: Dbc[m,q] = sum_p sel4[p,m]*dstage[p,q]
                osb = norm_state[(b, qb)]
                drb_ps = aux_ps.tile([128, QB], DT, name=f"drp{b}_{qb}", tag="aux")
                nc.tensor.matmul(
                    drb_ps[:], sel4_sb[:], dstage[:], start=True, stop=True
                )
                rec = npool.tile([128, QB], DT, name=f"rec{b}_{qb}", tag="rec")
                nc.vector.reciprocal_approx_fast(rec[:], drb_ps[:])
                of = opool.tile([128, QB], BF, name=f"of{b}_{qb}", tag="of",
                                bufs=4)
                nc.vector.tensor_mul(of[:], osb[:], rec[:])
                # gpsimd queue: no SP bulk-load backlog ahead of collectives
                if b < B - 1:
                    dst = a2a_in[b][:, :, qb * QS:(qb + 1) * QS]
                else:
                    dst = a2a_in3[qb // 2][:, :, (qb % 2) * QS:(qb % 2 + 1) * QS]
                nc.gpsimd.dma_start(
                    dst.rearrange("i p j -> p i j"),
                    of.rearrange("p (i j) -> p i j", i=NCORES),
                )

            def get_ofull(b):
                key = f"ofull{b}"
                if key not in qkv_state:
                    qkv_state[key] = qkpool.tile(
                        [128, CC, NQB * QS], BF, name=key, tag="ofull", bufs=3
                    )
                return qkv_state[key]

            def emit_a2a(b, half=None):
                if half is None:
                    cin, cout = a2a_in[b], a2a_out[b]
                else:
                    cin, cout = a2a_in3[half], a2a_out3[half]
                nc.gpsimd.collective_compute(
                    "AllToAll",
                    mybir.AluOpType.bypass,
                    ins=[cin.opt()],
                    outs=[cout.opt()],
                    replica_groups=[list(range(NCORES))],
                )
                ofull = get_ofull(b)
                # gpsimd queue: held until the AllToAll (same queue) completes
                if half is None:
                    nc.gpsimd.dma_start(
                        ofull[:], cout.rearrange("i p j -> p i j")
                    )
                else:
                    nc.gpsimd.dma_start(
                        ofull[:, :, half * 2 * QS:(half + 1) * 2 * QS],
                        cout.rearrange("i p j -> p i j"),
                    )

            def proj_oc(grp, oc, half=None):
                # half=None: full 256-col group; half=0/1: 128-col halves
                # (batch 3, so each half can run as its collective lands)
                j0 = 0 if half in (None, 0) else 2 * QS
                w = NQB * QS if half is None else 2 * QS

                def emit():
                    ofull = qkv_state[f"ofull{grp}"]
                    pps = aux_ps.tile([128, QB], DT, name=f"pp{grp}_{oc}", tag="aux")
                    for cc in range(CC):
                        nc.tensor.matmul(
                            pps[:, 0:w],
                            wp_sb[:, cc, oc * 128:(oc + 1) * 128],
                            ofull[:, cc, j0:j0 + w],
                            start=cc == 0,
                            stop=cc == CC - 1,
                        )
                    fo = npool.tile([128, NQB * QS], BF, name=f"fo{grp}_{oc}", tag="fo")
                    nc.vector.tensor_scalar_add(
                        fo[:, 0:w], pps[:, 0:w], bias_sb[:, oc:oc + 1]
                    )
                    nc.sync.dma_start(
                        out_ext[oc * 128:(oc + 1) * 128,
                                grp * NQB * QS + j0:grp * NQB * QS + j0 + w],
                        fo[:, 0:w],
                    )

                return emit

            # ---- filler machinery (scan-based: deadlines are enforced
            # across the whole queue, pacing picks the first eligible unit)
            fillers = []   # list of dicts; emitted entries are removed
            spent = [0.0]

            def enqueue(emit, cost, deadline, eligible=0):
                fillers.append(
                    dict(emit=emit, cost=cost, deadline=deadline, eligible=eligible)
                )

            def run_filler(f):
                f["emit"]()
                spent[0] += f["cost"]

            def pull_deadlines(i):
                rest = []
                for f in fillers:
                    if f["deadline"] <= i:
                        run_filler(f)
                    else:
                        rest.append(f)
                fillers[:] = rest

            def pace(i):
                target = (i + 1) * FILLER_BUDGET
                while True:
                    pick = None
                    for f in fillers:
                        if f["eligible"] <= i:
                            pick = f
                            break
                    if pick is None or spent[0] + pick["cost"] * 0.5 > target:
                        break
                    run_filler(pick)
                    fillers.remove(pick)

            def drain_all_fillers():
                for f in list(fillers):
                    run_filler(f)
                fillers[:] = []

            def enqueue_qkv(b, eligible):
                """Queue batch b's QKV units (minus any prologue prefix).
                Eligibility is staged to match DMA landing order: kb lands
                first, then xb half 0, then xb half 1."""
                nkc = nkcs[b]
                e_k = eligible
                e_v = eligible + 1
                e_q01 = eligible + 5
                e_q23 = eligible + 11
                if b == 0:
                    e_k = e_v = e_q01 = e_q23 = 0
                for bi, (pos, w) in enumerate(kblocks[b]):
                    if b == 0 and bi == 0:
                        continue
                    first_kc = pos // KCH
                    enqueue(k_unit(b, pos, w), 1730,
                            idx_of[(b, 0, min(first_kc, nkc - 1))], e_k)
                for rb in range(NQB):
                    if b == 0 and rb == 0:
                        continue
                    enqueue(q_unit(b, rb), 1730, idx_of[(b, rb, 0)],
                            e_q01 if rb < 2 else e_q23)
                for rc in range(nkc):
                    if b == 0 and rc < 2:
                        continue
                    enqueue(v_unit(b, rc), 460, idx_of[(b, 0, rc)] + LAG, e_v)

            def enqueue_proj(grp, eligible, hold_back=0):
                # ofull ring is 3 deep: group g's units must all be emitted
                # before ofull(g+3) is allocated early in batch g+3
                if grp + 3 < B:
                    dl = batch_start[grp + 3] + 4
                else:
                    dl = len(slots) + 10 ** 6
                held = []
                for oc in range(CC):
                    u = dict(emit=proj_oc(grp, oc), cost=950,
                             deadline=dl, eligible=eligible)
                    if oc >= CC - hold_back:
                        held.append(u)
                    else:
                        fillers.append(u)
                return held

            # ---- prologue: batch 0 prefix while DMAs land. DMA order: wk,
            # kb block0 (emitted above via load_kb0 inside emit order below),
            # wq, xb block0, wv, small consts, then the bulk.
            load_kb(0, 0)
            nc.sync.dma_start(wq_sb[:], wq.rearrange("(cc p) m -> p cc m", p=128))
            load_xbq(0, 0)
            nc.sync.dma_start(wv_sb[:], wv.rearrange("(cc p) m -> p cc m", p=128))
            nc.sync.dma_start(bias_sb[:], bvec[:])
            nc.sync.dma_start(mb_sb[:], mb[:])
            nc.sync.dma_start(sel4_sb[:], sel4[:])
            load_kb(0, 1)
            for rb in range(1, NQB):
                load_xbq(0, rb)
            make_qkv_tiles(0)
            pos0, w0 = kblocks[0][0]
            k_unit(0, pos0, w0)()
            q_unit(0, 0)()
            v_unit(0, 0)()
            v_unit(0, 1)()
            enqueue_qkv(0, eligible=0)

            # ---- main slot loop
            pend_normb = []   # (due_idx, b, qb)
            held_tail = []    # proj units reserved as tail filler

            def flush_normb(i):
                while pend_normb and pend_normb[0][0] <= i:
                    _, bb, qq = pend_normb.pop(0)
                    norm_b(bb, qq)
                    if bb == B - 1 and qq == 1:
                        # batch 3 first half: collective + its proj halves
                        # flow into the remaining batch-3 slots
                        emit_a2a(bb, half=0)
                        for oc in range(CC):
                            enqueue(proj_oc(bb, oc, half=0), 700,
                                    len(slots) + 10 ** 6, i + 14)
                    elif qq == NQB - 1 and bb < B - 1:
                        emit_a2a(bb)
                        # proj for batch bb: eligible once its AllToAll has
                        # had ~17us to land; batch 2 reserves four units to
                        # cover the final collective in the tail
                        hb = 4 if bb == B - 2 else 0
                        held_tail.extend(
                            enqueue_proj(bb, eligible=i + 18, hold_back=hb)
                        )

            for i, (b, qb, kc) in enumerate(slots):
                # next batch's loads start early; collective inputs are on
                # the gpsimd queue so they never sit behind these on SP
                if i - batch_start[b] == 2 and b + 1 < B:
                    emit_xb_load(b + 1)
                    make_qkv_tiles(b + 1)
                    # batch 1's loads queue behind the whole prologue on SP
                    enqueue_qkv(b + 1, eligible=i + (17 if b == 0 else 9))
                if i == 25:
                    # wp (2MB) queued after batch 1's loads; needed by
                    # proj(0) around the end of batch 1
                    nc.sync.dma_start(
                        wp_sb[:], wp.rearrange("(cc p) m -> p cc m", p=128)
                    )
            pend_d = []

            def after_av(bb, qq, kk, due):
                # emit D steps in adjacent parity pairs: the 4 M=1 matmuls
                # occupy disjoint 32-col groups and run concurrently
                pend_d.append((bb, qq, kk))
                if len(pend_d) == 2 or kk == nkcs[bb] - 1:
                    for (b2, q2, k2) in pend_d:
                        d_step(b2, q2, k2)
                    pend_d.clear()
                if kk == nkcs[bb] - 1:
                    norm_a(bb, qq)
                    pend_normb.append((due, bb, qq))

            for i, (b, qb, kc) in enumerate(slots):
                if qb == 0 and kc == 0 and b + 1 < B:
                    emit_xb_load(b + 1)
                    make_qkv_tiles(b + 1)
                    enqueue_qkv(b + 1, eligible=i + 8)
                pull_deadlines(i)
                flush_normb(i)
                se_step(b, qb, kc)
                pace(i)
                j = i - LAG
                if j >= 0:
                    bb, qq, kk = slots[j]
                    av_step(bb, qq, kk)
                    after_av(bb, qq, kk, i + 3)

            # ---- drain: remaining av/d, then the tail
            n_end = len(slots)
            for j in range(n_end - LAG, n_end):
                bb, qq, kk = slots[j]
                av_step(bb, qq, kk)
                after_av(bb, qq, kk, 10 ** 9)
            # one reserved proj unit covers the final reciprocal latency
            if held_tail:
                held_tail.pop(0)["emit"]()
            while pend_normb:
                _, bb, qq = pend_normb.pop(0)
                norm_b(bb, qq)
            emit_a2a(B - 1, half=1)
            # reserved proj(2) units and any leftover fillers (including
            # batch 3's first-half proj) execute during the final collective
            for u in held_tail:
                u["emit"]()
            drain_all_fillers()
            for oc in range(CC):
                proj_oc(B - 1, oc, half=1)()

    nc.compile()
    return nc


def _prep_inputs(x, Wqkv, Wproj, bproj, mask, nkcs):
    x = np.asarray(x, dtype=np.float32)
    Wqkv = np.asarray(Wqkv, dtype=np.float32)
    Wproj = np.asarray(Wproj, dtype=np.float32)
    bproj = np.asarray(bproj, dtype=np.float32)
    mask = np.asarray(mask)
    nks = [nkc * KCH for nkc in nkcs]
    koffs = np.concatenate([[0], np.cumsum(nks)]).astype(int)
    moffs = np.concatenate([[0], np.cumsum(nkcs)]).astype(int)
    totk = int(koffs[-1])
    totkc = int(moffs[-1])

    x2 = x.reshape(ROWS, C)
    xT = np.ascontiguousarray(x2.T).astype(NPBF)
    # compacted K/V tokens: unmasked columns per batch, zero-padded to nk_b
    xTk = np.zeros((C, totk), dtype=NPBF)
    mbias = np.full((totk,), np.float32(MASK_BIAS), dtype=np.float32)
    for b in range(B):
        idx = np.nonzero(mask[b] == 0)[0]
        cnt = len(idx)
        xTk[:, koffs[b]: koffs[b] + cnt] = xT[:, b * N + idx]
        mbias[koffs[b]: koffs[b] + cnt] = 0.0
    mb_arr = np.zeros((128, totkc), dtype=np.float32)
    for b in range(B):
        blk = mbias[koffs[b]:koffs[b + 1]].reshape(nkcs[b], 128).T
        mb_arr[:, moffs[b]:moffs[b + 1]] = blk

    wp_bf = Wproj.astype(NPBF)
    bias_r = np.ascontiguousarray(bproj.reshape(CC, 128).T).astype(np.float32)
    sel4 = np.zeros((128, 128), np.float32)
    sel4[0, 0:64] = 1.0
    sel4[64, 0:64] = 1.0
    sel4[32, 64:128] = 1.0
    sel4[96, 64:128] = 1.0

    in_maps = []
    for c in range(NCORES):
        cols = slice(c * CPC, (c + 1) * CPC)
        in_maps.append(
            dict(
                xT=xT,
                xTk=xTk,
                wq=np.ascontiguousarray(Wqkv[:, cols]).astype(NPBF),
                wk=np.ascontiguousarray(Wqkv[:, C:][:, cols]).astype(NPBF),
                wv=np.ascontiguousarray(Wqkv[:, 2 * C:][:, cols]).astype(NPBF),
                wp=wp_bf,
                bvec=bias_r,
                mb=mb_arr,
                sel4=sel4,
            )
        )
    return in_maps


def kernel(x, Wqkv, Wproj, bproj, mask):
    global LAST_RESULTS
    mask = np.asarray(mask)
    cnts = (mask == 0).sum(axis=1)
    nkcs = tuple(max(1, int(-(-c // KCH))) for c in cnts)
    if nkcs not in _CACHE:
        _CACHE[nkcs] = _build(nkcs)
    nc = _CACHE[nkcs]
    in_maps = _prep_inputs(x, Wqkv, Wproj, bproj, mask, nkcs)
    res = run_bass_kernel_spmd(nc, in_maps, list(range(NCORES)))
    LAST_RESULTS = res
    out = np.empty((ROWS, C), dtype=np.float32)
    for c in range(NCORES):
        oT = np.asarray(res.results[c]["out"], dtype=np.float32)
        for b in range(B):
            blk = oT[:, b * NQB * QS:(b + 1) * NQB * QS].reshape(C, NQB, QS)
            for qb in range(NQB):
                r0 = b * N + qb * QB + c * QS
                out[r0:r0 + QS, :] = blk[:, qb, :].T
    return out.reshape(B, N, C)
